# revision 41
# baseline (speedup 1.0000x reference)
"""Trainium2 Bass kernel for nn_Downsample_Spa: sigma-conv + gaussian unfold downsample.

Math (per batch image, one NeuronCore each; batch of 8 -> 8 cores):
  xp = reflect_pad(x)                                  # [64,130,130]
  sigma[o,p] = clamp(BN(conv3x3(xp))[o,p], 1e-4)       # at stride-2 positions p only
  graw[o,p]  = exp(-0.5*d2[o]/sigma^2 - ln64) / sigma  # /64 guards fp16 range; cancels in the ratio
  out[c,p]   = sum_o graw[o,p]*xp[c,p+off(o)] / sum_o graw[o,p]

Design:
 - partitions = (row-half hh, channel c) = 128; host pre-pads (reflect), fp16, and
   parity-splits columns into TWO planes (w=2j / w=2j+1); taps at w=2j+2 read
   plane0 at col+1 (still step-1). Input 2.2MB/core.
 - conv: 9 accumulating fp16 matmuls per 512-position block, block-diagonal
   weights (M=18 = both row halves). All 4 conv blocks run contiguously up front
   (a dense PE stretch holds the HAM clock-gate at K=8/8; cold PE halves matmul
   rate); sigma f32 in PSUM.
 - sigma clamp: block 0 on the then-idle DVE (tensor_scalar, shortest latency);
   blocks 1-3 on ACT as Relu(v + bias - eps) + eps == max(v + bias, eps), which
   keeps the saturated DVE out of the clamp. Per-block sc tiles for blocks 0/1
   (tile-granular deps would stall block 0's g pipe on block 1's conv).
 - g pipeline (per block for 0/1, per half for 2/3): custom-DVE fast reciprocal,
   ACT Square + Exp (per-partition scale/bias), DVE mult -> gb bf16.
 - unfold per block: one-hot bf16 matmuls broadcast gb to the 128 partitions
   (3-tap PSUM rowgroups + Srep sum matmul); ACT copies rowgroups to one fp16 gc
   tile; 9 all-fp16 tap products into two 4-slot tiles -> one 2x-mode t4 add
   (contiguous tiles; interleaved slots would drop DVE to 1x), t2, then
   t1/+center/*1/Srep offloaded to gpsimd on blocks 0-2; fp16 out DMA per block
   (host converts to f32).
 - PE warm-up matmuls on a memset scratch tile (no DMA dependency, starts right
   after the engine preamble); input chunks FIFO on the sync hw queue so chunk 0
   gets the full queue rate; small consts ride the scalar hw queue (the gpsimd
   software-DGE queue has multi-us latency; win's 324B/partition packets are
   slow, so it goes first on its queue).
"""

import os
import sys

import numpy as np

if "/opt/trn_rl_repo" not in sys.path:
    sys.path.insert(0, "/opt/trn_rl_repo")

K = 3
BN_EPS = 1e-5
SIGMA_MIN = 1e-4
GSCALE_LN = float(np.log(64.0))   # graw scaled by 1/64 (folded into exp bias)
N, C, H, W = 8, 64, 128, 128
HO = WO = 64
HH = 2
RS = 65                  # padded-row slots per partition-half
HOC = 32
NBLK = 4
BR = HOC // NBLK         # 8 output rows per block
NPOS = BR * WO           # 512
NP2 = 2 * NPOS
PL = 2                   # x col-parity planes: w=2j / w=2j+1
JW = 66                  # j slots per plane (65 used, 66 for alignment)
CR = 17                  # rows per DMA chunk tile (16 + 1 overlap)

# f32 consts tensor columns
_D2 = 0                  # -0.5*d2[o] per (hh,o)
_BC = 1                  # bn_bias - sigma_min
_LB = 2                  # exp bias: constant -ln(64) per partition
_EPS = 3                 # sigma clamp floor
_NCC = 4

_STATE = {}


def _build_consts(conv_w, bn_gamma, bn_beta, bn_mean, bn_var):
    s = (bn_gamma / np.sqrt(bn_var + BN_EPS)).astype(np.float32)
    wf = conv_w.astype(np.float32) * s[:, None, None, None]           # [9,64,3,3]
    bias = (bn_beta - bn_mean * s).astype(np.float32)

    cst = np.zeros((18, _NCC), np.float32)
    d2 = np.array([(kk // 3 - 1) ** 2 + (kk % 3 - 1) ** 2 for kk in range(9)], np.float32)
    for hh in range(HH):
        cst[hh * 9:hh * 9 + 9, _D2] = -0.5 * d2
        cst[hh * 9:hh * 9 + 9, _BC] = bias - SIGMA_MIN
        cst[hh * 9:hh * 9 + 9, _LB] = -GSCALE_LN
        cst[hh * 9:hh * 9 + 9, _EPS] = SIGMA_MIN

    # conv weights, block-diagonal per tap: win[k=hh*64+c, tap*18 + hh*9+o]
    win = np.zeros((128, 9 * 18), np.float16)
    for tap in range(9):
        i, j = tap // 3, tap % 3
        for hh in range(HH):
            win[hh * 64:hh * 64 + 64, tap * 18 + hh * 9:tap * 18 + hh * 9 + 9] = \
                wf[:, :, i, j].T.astype(np.float16)

    # one-hot / ones broadcast weights: gin[k=hh*9+o, tap*128 + hh*64+c]
    import ml_dtypes
    gin = np.zeros((18, 10 * 128), ml_dtypes.bfloat16)
    for hh in range(HH):
        gin[hh * 9:hh * 9 + 9, 9 * 128 + hh * 64:9 * 128 + hh * 64 + 64] = 1.0
        for tap in range(9):
            gin[hh * 9 + tap, tap * 128 + hh * 64:tap * 128 + hh * 64 + 64] = 1.0
    return cst, win, gin


def _build_bass(for_sim=False):
    import concourse.bass as bass
    import concourse.tile as tile
    from concourse import mybir

    f32 = mybir.dt.float32
    f16 = mybir.dt.float16
    bf16 = mybir.dt.bfloat16
    MULT = mybir.AluOpType.mult
    ADD = mybir.AluOpType.add
    MAX = mybir.AluOpType.max
    AF = mybir.ActivationFunctionType

    if for_sim:
        nc = bass.Bass("TRN2", target_bir_lowering=False, detect_race_conditions=False)
    else:
        from concourse import bacc
        nc = bacc.Bacc()
    xin = nc.dram_tensor("xin", [128, RS, PL, JW], f16, kind="ExternalInput")
    cin = nc.dram_tensor("cin", [18, _NCC], f32, kind="ExternalInput")
    win = nc.dram_tensor("win", [128, 9 * 18], f16, kind="ExternalInput")
    gin = nc.dram_tensor("gin", [18, 10 * 128], bf16, kind="ExternalInput")
    out = nc.dram_tensor("out", [128, HOC, WO], f16, kind="ExternalOutput")

    with tile.TileContext(nc) as tc:
        from contextlib import ExitStack
        with ExitStack() as ctx:
            big = ctx.enter_context(tc.tile_pool(name="big", bufs=1))
            gsb = ctx.enter_context(tc.tile_pool(name="gsb", bufs=3))
            y_p = ctx.enter_context(tc.tile_pool(name="y", bufs=3))
            ps_s = ctx.enter_context(tc.tile_pool(name="ps_s", bufs=2, space="PSUM"))
            ps_g = ctx.enter_context(tc.tile_pool(name="ps_g", bufs=3, space="PSUM"))

            # warm-up operand: memset scratch, so the PE can start before any DMA
            scr = big.tile([128, 162], f16)
            nc.vector.memset(scr[:], 0.0)

            # ---- input DMAs over both hw queues; chunk 0 first ----
            ws = big.tile([128, 9 * 18], f16)
            cs = big.tile([18, _NCC], f32)
            gs = big.tile([18, 10 * 128], bf16)
            # 4 input chunks FIFO on the sync hw queue: chunk 0 gets the full
            # queue rate and lands first; tiny consts ride the scalar queue
            xsk = []
            for b in range(NBLK):
                xs = big.tile([128, CR, PL, JW], f16, tag=f"xs{b}")
                xsk.append(xs)
            nc.scalar.dma_start(out=ws[:], in_=win[:])
            nc.sync.dma_start(out=xsk[0][:], in_=xin[:, 0:CR, :, :])
            nc.scalar.dma_start(out=cs[:], in_=cin[:])
            nc.scalar.dma_start(out=gs[:], in_=gin[:])
            nc.sync.dma_start(out=xsk[1][:], in_=xin[:, 16:16 + CR, :, :])
            nc.sync.dma_start(out=xsk[2][:], in_=xin[:, 32:32 + CR, :, :])
            nc.sync.dma_start(out=xsk[3][:], in_=xin[:, 48:48 + CR, :, :])

            def xtap(tap, blk):
                # [128, 8, 64] fp16 step-1 view for tap (i,j) in block blk
                i, j = tap // 3, tap % 3
                if j < 2:
                    return xsk[blk][:, i:i + 2 * BR - 1:2, j, 0:WO]
                return xsk[blk][:, i:i + 2 * BR - 1:2, 0, 1:WO + 1]

            def xpair(i, blk):
                # [128, 2, 8, 64] planes (w=2j, 2j+1) for tap row i
                return xsk[blk][:, i:i + 2 * BR - 1:2, 0:PL, 0:WO].transpose([0, 2, 1, 3])

            # ---- PE warm-up on scratch (~3.4us) so HAM un-throttles pre-conv ----
            wu = ps_s.tile([18, NPOS], f32, tag="sig")
            for _ in range(30):
                nc.tensor.matmul(wu[:, 0:162], scr[:, 0:18], scr[:, 0:162],
                                 start=True, stop=True)

            # ---- conv: per-block sigma [18,512] in PSUM, clamped on ACT.
            # Blocks 0/1 get their own sc tiles (tile-granular deps: a shared
            # tile would stall block-0's g pipe on block-1's conv); blocks 2/3
            # share a [18,1024] half tile ----
            def conv_mm(blk):
                sig = ps_s.tile([18, NPOS], f32, tag="sig")
                for tap in range(9):
                    nc.tensor.matmul(
                        sig[:],
                        ws[:, tap * 18:(tap + 1) * 18],
                        xtap(tap, blk),
                        start=(tap == 0), stop=(tap == 8),
                    )
                return sig

            def clamp_dve(sig):
                # block 0 clamps on the (still idle) DVE: shortest latency
                scb = gsb.tile([18, NPOS], f32, tag="scb")
                nc.vector.tensor_scalar(out=scb[:], in0=sig[:],
                                        scalar1=cs[:, _BC:_BC + 1],
                                        scalar2=float(SIGMA_MIN),
                                        op0=ADD, op1=MAX)
                return scb

            def clamp_act(sig, dst):
                # clamp on ACT, not DVE: max(v, f) == Relu(v - f) + f exactly
                sp = gsb.tile([18, NPOS], f32, tag="sp")
                nc.scalar.activation(out=sp[:], in_=sig[:], func=AF.Relu,
                                     bias=cs[:, _BC:_BC + 1])
                nc.scalar.activation(out=dst, in_=sp[:], func=AF.Identity,
                                     bias=cs[:, _EPS:_EPS + 1])

            def inv_emit(scv):
                inv = gsb.tile([18, NPOS], f32, tag="inv")
                nc.vector.reciprocal_approx_fast(out=inv[:], in_=scv)
                return inv

            def qe_emit(inv):
                qt = gsb.tile([18, NPOS], f32, tag="qt")
                nc.scalar.activation(out=qt[:], in_=inv[:], func=AF.Square)
                et = gsb.tile([18, NPOS], f32, tag="et")
                nc.scalar.activation(out=et[:], in_=qt[:], func=AF.Exp,
                                     scale=cs[:, _D2:_D2 + 1],
                                     bias=cs[:, _LB:_LB + 1])
                return et

            def gbm_emit(inv, et):
                gb = gsb.tile([18, NPOS], bf16, tag="gb")
                nc.vector.tensor_tensor(out=gb[:], in0=et[:], in1=inv[:], op=MULT)
                return gb

            def unfold_emit(blk, gbs, late):
                # gbs: [18, 512] slice (this block's g, taps on partitions)
                # 2-tap PSUM rowgroups (2 banks x 3 bufs instead of 3 banks x 2):
                # deeper broadcast/copy pipelining, finer product dependencies.
                # Srep (ones) shares the first group tile and is read from PSUM.
                gp = ps_g.tile([128, 2, NPOS], f32, tag="grep")
                nc.tensor.matmul(gp[:, 1], gs[:, 9 * 128:10 * 128], gbs,
                                 start=True, stop=True)
                nc.tensor.matmul(gp[:, 0], gs[:, 8 * 128:9 * 128], gbs,
                                 start=True, stop=True)
                rr = y_p.tile([128, BR, WO], f32, tag="rr")
                nc.vector.reciprocal_approx_fast(out=rr[:], in_=gp[:, 1])
                gc8 = y_p.tile([128, BR, WO], f16, tag="gc8")
                nc.scalar.activation(out=gc8[:], in_=gp[:, 0], func=AF.Copy)
                gct = []
                for g in range(4):  # tap pairs (0,1), (2,3), (4,5), (6,7)
                    g2p = ps_g.tile([128, 2, NPOS], f32, tag="grep")
                    for k in range(2):
                        tap = 2 * g + k
                        nc.tensor.matmul(g2p[:, k], gs[:, tap * 128:(tap + 1) * 128],
                                         gbs, start=True, stop=True)
                    gcg = y_p.tile([128, 2, BR, WO], f16, tag=f"gd{g}")
                    nc.scalar.activation(out=gcg[:], in_=g2p[:], func=AF.Copy)
                    gct.append(gcg)

                # products, all fp16: ytA = [r0j0, r0j1, r0j2, r1j0],
                # ytB = [r2j0, r2j1, r2j2, r1j2], yC = center
                ytA = y_p.tile([128, 4, BR, WO], f16, tag="ytA")
                ytB = y_p.tile([128, 4, BR, WO], f16, tag="ytB")
                yC = y_p.tile([128, BR, WO], f16, tag="yC")
                nc.vector.tensor_tensor(out=ytA[:, 0:2], in0=xpair(0, blk),
                                        in1=gct[0][:], op=MULT)
                nc.vector.tensor_tensor(out=ytA[:, 2], in0=xtap(2, blk),
                                        in1=gct[1][:, 0], op=MULT)
                nc.vector.tensor_tensor(out=ytA[:, 3], in0=xtap(3, blk),
                                        in1=gct[1][:, 1], op=MULT)
                nc.vector.tensor_tensor(out=yC[:], in0=xtap(4, blk),
                                        in1=gct[2][:, 0], op=MULT)
                nc.vector.tensor_tensor(out=ytB[:, 3], in0=xtap(5, blk),
                                        in1=gct[2][:, 1], op=MULT)
                nc.vector.tensor_tensor(out=ytB[:, 0:2], in0=xpair(2, blk),
                                        in1=gct[3][:], op=MULT)
                nc.vector.tensor_tensor(out=ytB[:, 2], in0=xtap(8, blk),
                                        in1=gc8[:], op=MULT)

                # pair tree (fp16, 2x-mode contiguous tiles) + center + normalize
                t4 = y_p.tile([128, 4, BR, WO], f16, tag="t4")
                nc.vector.tensor_tensor(out=t4[:], in0=ytA[:], in1=ytB[:], op=ADD)
                eng1 = nc.vector if late else nc.gpsimd
                t2 = y_p.tile([128, 2, BR, WO], f16, tag="t2")
                nc.vector.tensor_tensor(out=t2[:], in0=t4[:, 0:2], in1=t4[:, 2:4], op=ADD)
                t1 = y_p.tile([128, BR, WO], f16, tag="t1")
                eng1.tensor_tensor(out=t1[:], in0=t2[:, 0], in1=t2[:, 1], op=ADD)
                tC = y_p.tile([128, BR, WO], f16, tag="tC")
                eng1.tensor_tensor(out=tC[:], in0=t1[:], in1=yC[:], op=ADD)
                o16 = y_p.tile([128, BR, WO], f16, tag="o16")
                eng1.tensor_tensor(out=o16[:], in0=tC[:], in1=rr[:], op=MULT)
                nc.sync.dma_start(out=out[:, BR * blk:BR * (blk + 1), :], in_=o16[:])

            # per-block g pipes, emitted right after each block's clamp; blocks
            # 0/1 clamp on the then-idle DVE (also fills the wait on ACT's
            # Square/Exp), 2/3 on ACT. gb multiplies are queued after both invs
            # so the in-order DVE queue never stalls waiting on ACT.
            sig0 = conv_mm(0)
            sc0 = clamp_dve(sig0)
            inv0 = inv_emit(sc0[:])
            et0 = qe_emit(inv0)
            sig1 = conv_mm(1)
            sc1 = clamp_dve(sig1)
            inv1 = inv_emit(sc1[:])
            et1 = qe_emit(inv1)
            gb0 = gbm_emit(inv0, et0)
            gb1 = gbm_emit(inv1, et1)
            sig2 = conv_mm(2)
            sc2 = gsb.tile([18, NPOS], f32, tag="scb")
            clamp_act(sig2, sc2[:])
            inv2 = inv_emit(sc2[:])
            et2 = qe_emit(inv2)
            gb2 = gbm_emit(inv2, et2)
            # unfold 0 before conv3 in the PE queue: conv3 waits on the last
            # input chunk, and block 0's broadcasts should not wait behind it
            unfold_emit(0, gb0[:], late=False)
            sig3 = conv_mm(3)
            sc3 = gsb.tile([18, NPOS], f32, tag="scb")
            clamp_act(sig3, sc3[:])
            inv3 = inv_emit(sc3[:])
            et3 = qe_emit(inv3)
            gb3 = gbm_emit(inv3, et3)
            unfold_emit(1, gb1[:], late=False)
            unfold_emit(2, gb2[:], late=False)
            unfold_emit(3, gb3[:], late=True)

    if not for_sim and not nc.is_finalized():
        nc.finalize()
    return nc


def _prep_inputs(x, conv_w, bn_gamma, bn_beta, bn_mean, bn_var):
    cst, win, gin = _build_consts(conv_w, bn_gamma, bn_beta, bn_mean, bn_var)
    xp = np.pad(np.asarray(x, np.float32), ((0, 0), (0, 0), (1, 1), (1, 1)),
                mode="reflect").astype(np.float16)                    # [8,64,130,130]
    in_maps = []
    for n in range(N):
        xc = np.concatenate([xp[n, :, 0:RS, :], xp[n, :, 64:64 + RS, :]], axis=0)
        xpl = np.zeros((128, RS, PL, JW), np.float16)
        xpl[:, :, 0, 0:65] = xc[:, :, 0:130:2]
        xpl[:, :, 1, 0:65] = xc[:, :, 1:130:2]
        in_maps.append({"xin": xpl, "cin": cst, "win": win, "gin": gin})
    return in_maps


def _gather(results):
    out = np.empty((N, C, HO, WO), np.float32)
    for n in range(N):
        d = np.asarray(results[n]["out"], np.float32)
        out[n, :, 0:HOC, :] = d[0:64]
        out[n, :, HOC:, :] = d[64:128]
    return out


def _enable_axon_trace():
    """Register the NTFF profile hook that this image's antenv lacks."""
    if _STATE.get("trace_hooked"):
        return
    import types
    import antenv
    from concourse import bass_utils
    mod = types.ModuleType("antenv.axon_hooks")
    mod._hook = None
    mod.set_axon_ntff_profile_hook = lambda h: setattr(mod, "_hook", h)
    mod.get_axon_ntff_profile_hook = lambda: mod._hook
    sys.modules["antenv.axon_hooks"] = mod
    antenv.axon_hooks = mod
    from trn_agent_boot.trn_boot import _ntff_profile_via_ctypes
    mod._hook = _ntff_profile_via_ctypes("/opt/axon/libaxon_pjrt.so")
    bass_utils.upload_artifacts = lambda tmpdir: tmpdir
    _STATE["trace_hooked"] = True


def run(x, conv_w, bn_gamma, bn_beta, bn_mean, bn_var, trace=False):
    from concourse.bass_utils import run_bass_kernel_spmd
    if trace:
        _enable_axon_trace()
    if "nc" not in _STATE:
        _STATE["nc"] = _build_bass()
    in_maps = _prep_inputs(x, conv_w, bn_gamma, bn_beta, bn_mean, bn_var)
    res = run_bass_kernel_spmd(_STATE["nc"], in_maps, list(range(N)), trace=trace)
    _STATE["last"] = res
    return _gather(res.results)


def kernel(x, conv_w, bn_gamma, bn_beta, bn_mean, bn_var):
    return run(x, conv_w, bn_gamma, bn_beta, bn_mean, bn_var,
               trace=bool(int(os.environ.get("KERNEL_TRACE", "0"))))


# revision 43
# speedup vs baseline: 1.0026x; 1.0026x over previous
"""Trainium2 Bass kernel for nn_Downsample_Spa: sigma-conv + gaussian unfold downsample.

Math (per batch image, one NeuronCore each; batch of 8 -> 8 cores):
  xp = reflect_pad(x)                                  # [64,130,130]
  sigma[o,p] = clamp(BN(conv3x3(xp))[o,p], 1e-4)       # at stride-2 positions p only
  graw[o,p]  = exp(-0.5*d2[o]/sigma^2 - ln64) / sigma  # /64 guards fp16 range; cancels in the ratio
  out[c,p]   = sum_o graw[o,p]*xp[c,p+off(o)] / sum_o graw[o,p]

Design:
 - partitions = (row-half hh, channel c) = 128; host pre-pads (reflect), fp16, and
   parity-splits columns into TWO planes (w=2j / w=2j+1); taps at w=2j+2 read
   plane0 at col+1 (still step-1). Input 2.2MB/core.
 - conv: 9 accumulating fp16 matmuls per 512-position block, block-diagonal
   weights (M=18 = both row halves). All 4 conv blocks run contiguously up front
   (a dense PE stretch holds the HAM clock-gate at K=8/8; cold PE halves matmul
   rate); sigma f32 in PSUM.
 - sigma clamp: blocks 0/1 on the then-idle DVE (tensor_scalar, shortest
   latency, fills the wait on ACT's Square/Exp); blocks 2/3 on ACT as
   Relu(v + bias - eps) + eps == max(v + bias, eps), off the saturated DVE.
 - g pipeline PER BLOCK, each emitted right after its own clamp (dependencies
   are tile-granular and engine queues are in-order, so coarser tiles or other
   emission orders stall earlier blocks on later convs): custom-DVE fast
   reciprocal, ACT Square + Exp (per-partition scale/bias), DVE mult -> gb bf16,
   with gb multiplies queued after both invs so DVE never stalls on ACT.
 - unfold per block: one-hot bf16 matmuls broadcast gb to the 128 partitions
   into 2-tap PSUM rowgroups (2 banks x 3 bufs; Srep ones-sum shares the first
   group and is read from PSUM); ACT copies each rowgroup to its own fp16 tile;
   9 all-fp16 tap products into two 4-slot tiles -> one 2x-mode t4 add
   (contiguous tiles; interleaved slots would drop DVE to 1x), t2, then
   t1/+center/*1/Srep offloaded to gpsimd on blocks 0-2; fp16 out DMA per block
   (host converts to f32). unfold 0 is emitted before conv3 so block 0's
   broadcasts do not queue behind the last input chunk's DMA.
 - PE warm-up matmuls on a memset scratch tile (no DMA dependency, starts right
   after the engine preamble); input chunks FIFO on the sync hw queue so chunk 0
   gets the full queue rate; small consts ride the scalar hw queue (the gpsimd
   software-DGE queue has multi-us latency; win's 324B/partition packets are
   slow, so it goes first on its queue).
"""

import os
import sys

import numpy as np

if "/opt/trn_rl_repo" not in sys.path:
    sys.path.insert(0, "/opt/trn_rl_repo")

K = 3
BN_EPS = 1e-5
SIGMA_MIN = 1e-4
GSCALE_LN = float(np.log(64.0))   # graw scaled by 1/64 (folded into exp bias)
N, C, H, W = 8, 64, 128, 128
HO = WO = 64
HH = 2
RS = 65                  # padded-row slots per partition-half
HOC = 32
NBLK = 4
BR = HOC // NBLK         # 8 output rows per block
NPOS = BR * WO           # 512
NP2 = 2 * NPOS
PL = 2                   # x col-parity planes: w=2j / w=2j+1
JW = 66                  # j slots per plane (65 used, 66 for alignment)
CR = 17                  # rows per DMA chunk tile (16 + 1 overlap)

# f32 consts tensor columns
_D2 = 0                  # -0.5*d2[o] per (hh,o)
_BC = 1                  # bn_bias - sigma_min
_LB = 2                  # exp bias: constant -ln(64) per partition
_EPS = 3                 # sigma clamp floor
_NCC = 4

_STATE = {}


def _build_consts(conv_w, bn_gamma, bn_beta, bn_mean, bn_var):
    s = (bn_gamma / np.sqrt(bn_var + BN_EPS)).astype(np.float32)
    wf = conv_w.astype(np.float32) * s[:, None, None, None]           # [9,64,3,3]
    bias = (bn_beta - bn_mean * s).astype(np.float32)

    cst = np.zeros((18, _NCC), np.float32)
    d2 = np.array([(kk // 3 - 1) ** 2 + (kk % 3 - 1) ** 2 for kk in range(9)], np.float32)
    for hh in range(HH):
        cst[hh * 9:hh * 9 + 9, _D2] = -0.5 * d2
        cst[hh * 9:hh * 9 + 9, _BC] = bias - SIGMA_MIN
        cst[hh * 9:hh * 9 + 9, _LB] = -GSCALE_LN
        cst[hh * 9:hh * 9 + 9, _EPS] = SIGMA_MIN

    # conv weights, block-diagonal per tap: win[k=hh*64+c, tap*18 + hh*9+o]
    win = np.zeros((128, 9 * 18), np.float16)
    for tap in range(9):
        i, j = tap // 3, tap % 3
        for hh in range(HH):
            win[hh * 64:hh * 64 + 64, tap * 18 + hh * 9:tap * 18 + hh * 9 + 9] = \
                wf[:, :, i, j].T.astype(np.float16)

    # one-hot / ones broadcast weights: gin[k=hh*9+o, tap*128 + hh*64+c]
    import ml_dtypes
    gin = np.zeros((18, 10 * 128), ml_dtypes.bfloat16)
    for hh in range(HH):
        gin[hh * 9:hh * 9 + 9, 9 * 128 + hh * 64:9 * 128 + hh * 64 + 64] = 1.0
        for tap in range(9):
            gin[hh * 9 + tap, tap * 128 + hh * 64:tap * 128 + hh * 64 + 64] = 1.0
    return cst, win, gin


def _build_bass(for_sim=False):
    import concourse.bass as bass
    import concourse.tile as tile
    from concourse import mybir

    f32 = mybir.dt.float32
    f16 = mybir.dt.float16
    bf16 = mybir.dt.bfloat16
    MULT = mybir.AluOpType.mult
    ADD = mybir.AluOpType.add
    MAX = mybir.AluOpType.max
    AF = mybir.ActivationFunctionType

    if for_sim:
        nc = bass.Bass("TRN2", target_bir_lowering=False, detect_race_conditions=False)
    else:
        from concourse import bacc
        nc = bacc.Bacc()
    xin = nc.dram_tensor("xin", [128, RS, PL, JW], f16, kind="ExternalInput")
    cin = nc.dram_tensor("cin", [18, _NCC], f32, kind="ExternalInput")
    win = nc.dram_tensor("win", [128, 9 * 18], f16, kind="ExternalInput")
    gin = nc.dram_tensor("gin", [18, 10 * 128], bf16, kind="ExternalInput")
    out = nc.dram_tensor("out", [128, HOC, WO], f16, kind="ExternalOutput")

    with tile.TileContext(nc) as tc:
        from contextlib import ExitStack
        with ExitStack() as ctx:
            big = ctx.enter_context(tc.tile_pool(name="big", bufs=1))
            gsb = ctx.enter_context(tc.tile_pool(name="gsb", bufs=3))
            y_p = ctx.enter_context(tc.tile_pool(name="y", bufs=3))
            ps_s = ctx.enter_context(tc.tile_pool(name="ps_s", bufs=2, space="PSUM"))
            ps_g = ctx.enter_context(tc.tile_pool(name="ps_g", bufs=3, space="PSUM"))

            # warm-up operand: memset scratch, so the PE can start before any DMA
            scr = big.tile([128, 162], f16)
            nc.vector.memset(scr[:], 0.0)

            # ---- input DMAs over both hw queues; chunk 0 first ----
            ws = big.tile([128, 9 * 18], f16)
            cs = big.tile([18, _NCC], f32)
            gs = big.tile([18, 10 * 128], bf16)
            # 4 input chunks FIFO on the sync hw queue: chunk 0 gets the full
            # queue rate and lands first; tiny consts ride the scalar queue
            xsk = []
            for b in range(NBLK):
                xs = big.tile([128, CR, PL, JW], f16, tag=f"xs{b}")
                xsk.append(xs)
            nc.scalar.dma_start(out=ws[:], in_=win[:])
            nc.sync.dma_start(out=xsk[0][:], in_=xin[:, 0:CR, :, :])
            nc.scalar.dma_start(out=cs[:], in_=cin[:])
            nc.scalar.dma_start(out=gs[:], in_=gin[:])
            nc.sync.dma_start(out=xsk[1][:], in_=xin[:, 16:16 + CR, :, :])
            nc.sync.dma_start(out=xsk[2][:], in_=xin[:, 32:32 + CR, :, :])
            nc.sync.dma_start(out=xsk[3][:], in_=xin[:, 48:48 + CR, :, :])

            def xtap(tap, blk):
                # [128, 8, 64] fp16 step-1 view for tap (i,j) in block blk
                i, j = tap // 3, tap % 3
                if j < 2:
                    return xsk[blk][:, i:i + 2 * BR - 1:2, j, 0:WO]
                return xsk[blk][:, i:i + 2 * BR - 1:2, 0, 1:WO + 1]

            def xpair(i, blk):
                # [128, 2, 8, 64] planes (w=2j, 2j+1) for tap row i
                return xsk[blk][:, i:i + 2 * BR - 1:2, 0:PL, 0:WO].transpose([0, 2, 1, 3])

            # ---- PE warm-up on scratch (~3.4us) so HAM un-throttles pre-conv ----
            wu = ps_s.tile([18, NPOS], f32, tag="sig")
            for _ in range(27):
                nc.tensor.matmul(wu[:, 0:162], scr[:, 0:18], scr[:, 0:162],
                                 start=True, stop=True)

            # ---- conv: per-block sigma [18,512] in PSUM, clamped on ACT.
            # Blocks 0/1 get their own sc tiles (tile-granular deps: a shared
            # tile would stall block-0's g pipe on block-1's conv); blocks 2/3
            # share a [18,1024] half tile ----
            def conv_mm(blk):
                sig = ps_s.tile([18, NPOS], f32, tag="sig")
                for tap in range(9):
                    nc.tensor.matmul(
                        sig[:],
                        ws[:, tap * 18:(tap + 1) * 18],
                        xtap(tap, blk),
                        start=(tap == 0), stop=(tap == 8),
                    )
                return sig

            def clamp_dve(sig):
                # block 0 clamps on the (still idle) DVE: shortest latency
                scb = gsb.tile([18, NPOS], f32, tag="scb")
                nc.vector.tensor_scalar(out=scb[:], in0=sig[:],
                                        scalar1=cs[:, _BC:_BC + 1],
                                        scalar2=float(SIGMA_MIN),
                                        op0=ADD, op1=MAX)
                return scb

            def clamp_act(sig, dst):
                # clamp on ACT, not DVE: max(v, f) == Relu(v - f) + f exactly
                sp = gsb.tile([18, NPOS], f32, tag="sp")
                nc.scalar.activation(out=sp[:], in_=sig[:], func=AF.Relu,
                                     bias=cs[:, _BC:_BC + 1])
                nc.scalar.activation(out=dst, in_=sp[:], func=AF.Identity,
                                     bias=cs[:, _EPS:_EPS + 1])

            def inv_emit(scv):
                inv = gsb.tile([18, NPOS], f32, tag="inv")
                nc.vector.reciprocal_approx_fast(out=inv[:], in_=scv)
                return inv

            def qe_emit(inv):
                qt = gsb.tile([18, NPOS], f32, tag="qt")
                nc.scalar.activation(out=qt[:], in_=inv[:], func=AF.Square)
                et = gsb.tile([18, NPOS], f32, tag="et")
                nc.scalar.activation(out=et[:], in_=qt[:], func=AF.Exp,
                                     scale=cs[:, _D2:_D2 + 1],
                                     bias=cs[:, _LB:_LB + 1])
                return et

            def gbm_emit(inv, et):
                gb = gsb.tile([18, NPOS], bf16, tag="gb")
                nc.vector.tensor_tensor(out=gb[:], in0=et[:], in1=inv[:], op=MULT)
                return gb

            def unfold_emit(blk, gbs, late):
                # gbs: [18, 512] slice (this block's g, taps on partitions)
                # 2-tap PSUM rowgroups (2 banks x 3 bufs instead of 3 banks x 2):
                # deeper broadcast/copy pipelining, finer product dependencies.
                # Srep (ones) shares the first group tile and is read from PSUM.
                gp = ps_g.tile([128, 2, NPOS], f32, tag="grep")
                nc.tensor.matmul(gp[:, 1], gs[:, 9 * 128:10 * 128], gbs,
                                 start=True, stop=True)
                nc.tensor.matmul(gp[:, 0], gs[:, 8 * 128:9 * 128], gbs,
                                 start=True, stop=True)
                rr = y_p.tile([128, BR, WO], f32, tag="rr")
                nc.vector.reciprocal_approx_fast(out=rr[:], in_=gp[:, 1])
                gc8 = y_p.tile([128, BR, WO], f16, tag="gc8")
                nc.scalar.activation(out=gc8[:], in_=gp[:, 0], func=AF.Copy)
                gct = []
                for g in range(4):  # tap pairs (0,1), (2,3), (4,5), (6,7)
                    g2p = ps_g.tile([128, 2, NPOS], f32, tag="grep")
                    for k in range(2):
                        tap = 2 * g + k
                        nc.tensor.matmul(g2p[:, k], gs[:, tap * 128:(tap + 1) * 128],
                                         gbs, start=True, stop=True)
                    gcg = y_p.tile([128, 2, BR, WO], f16, tag=f"gd{g}")
                    nc.scalar.activation(out=gcg[:], in_=g2p[:], func=AF.Copy)
                    gct.append(gcg)

                # products, all fp16: ytA = [r0j0, r0j1, r0j2, r1j0],
                # ytB = [r2j0, r2j1, r2j2, r1j2], yC = center
                ytA = y_p.tile([128, 4, BR, WO], f16, tag="ytA")
                ytB = y_p.tile([128, 4, BR, WO], f16, tag="ytB")
                yC = y_p.tile([128, BR, WO], f16, tag="yC")
                nc.vector.tensor_tensor(out=ytA[:, 0:2], in0=xpair(0, blk),
                                        in1=gct[0][:], op=MULT)
                nc.vector.tensor_tensor(out=ytA[:, 2], in0=xtap(2, blk),
                                        in1=gct[1][:, 0], op=MULT)
                nc.vector.tensor_tensor(out=ytA[:, 3], in0=xtap(3, blk),
                                        in1=gct[1][:, 1], op=MULT)
                nc.vector.tensor_tensor(out=yC[:], in0=xtap(4, blk),
                                        in1=gct[2][:, 0], op=MULT)
                nc.vector.tensor_tensor(out=ytB[:, 3], in0=xtap(5, blk),
                                        in1=gct[2][:, 1], op=MULT)
                nc.vector.tensor_tensor(out=ytB[:, 0:2], in0=xpair(2, blk),
                                        in1=gct[3][:], op=MULT)
                nc.vector.tensor_tensor(out=ytB[:, 2], in0=xtap(8, blk),
                                        in1=gc8[:], op=MULT)

                # pair tree (fp16, 2x-mode contiguous tiles) + center + normalize
                t4 = y_p.tile([128, 4, BR, WO], f16, tag="t4")
                nc.vector.tensor_tensor(out=t4[:], in0=ytA[:], in1=ytB[:], op=ADD)
                eng1 = nc.vector if late else nc.gpsimd
                t2 = y_p.tile([128, 2, BR, WO], f16, tag="t2")
                nc.vector.tensor_tensor(out=t2[:], in0=t4[:, 0:2], in1=t4[:, 2:4], op=ADD)
                t1 = y_p.tile([128, BR, WO], f16, tag="t1")
                eng1.tensor_tensor(out=t1[:], in0=t2[:, 0], in1=t2[:, 1], op=ADD)
                tC = y_p.tile([128, BR, WO], f16, tag="tC")
                eng1.tensor_tensor(out=tC[:], in0=t1[:], in1=yC[:], op=ADD)
                o16 = y_p.tile([128, BR, WO], f16, tag="o16")
                eng1.tensor_tensor(out=o16[:], in0=tC[:], in1=rr[:], op=MULT)
                nc.sync.dma_start(out=out[:, BR * blk:BR * (blk + 1), :], in_=o16[:])

            # per-block g pipes, emitted right after each block's clamp; blocks
            # 0/1 clamp on the then-idle DVE (also fills the wait on ACT's
            # Square/Exp), 2/3 on ACT. gb multiplies are queued after both invs
            # so the in-order DVE queue never stalls waiting on ACT.
            sig0 = conv_mm(0)
            sc0 = clamp_dve(sig0)
            inv0 = inv_emit(sc0[:])
            et0 = qe_emit(inv0)
            sig1 = conv_mm(1)
            sc1 = clamp_dve(sig1)
            inv1 = inv_emit(sc1[:])
            et1 = qe_emit(inv1)
            gb0 = gbm_emit(inv0, et0)
            gb1 = gbm_emit(inv1, et1)
            sig2 = conv_mm(2)
            sc2 = gsb.tile([18, NPOS], f32, tag="scb")
            clamp_act(sig2, sc2[:])
            inv2 = inv_emit(sc2[:])
            et2 = qe_emit(inv2)
            gb2 = gbm_emit(inv2, et2)
            # unfold 0 before conv3 in the PE queue: conv3 waits on the last
            # input chunk, and block 0's broadcasts should not wait behind it
            unfold_emit(0, gb0[:], late=False)
            sig3 = conv_mm(3)
            sc3 = gsb.tile([18, NPOS], f32, tag="scb")
            clamp_act(sig3, sc3[:])
            inv3 = inv_emit(sc3[:])
            et3 = qe_emit(inv3)
            gb3 = gbm_emit(inv3, et3)
            unfold_emit(1, gb1[:], late=False)
            unfold_emit(2, gb2[:], late=False)
            unfold_emit(3, gb3[:], late=True)

    if not for_sim and not nc.is_finalized():
        nc.finalize()
    return nc


def _prep_inputs(x, conv_w, bn_gamma, bn_beta, bn_mean, bn_var):
    cst, win, gin = _build_consts(conv_w, bn_gamma, bn_beta, bn_mean, bn_var)
    xp = np.pad(np.asarray(x, np.float32), ((0, 0), (0, 0), (1, 1), (1, 1)),
                mode="reflect").astype(np.float16)                    # [8,64,130,130]
    in_maps = []
    for n in range(N):
        xc = np.concatenate([xp[n, :, 0:RS, :], xp[n, :, 64:64 + RS, :]], axis=0)
        xpl = np.zeros((128, RS, PL, JW), np.float16)
        xpl[:, :, 0, 0:65] = xc[:, :, 0:130:2]
        xpl[:, :, 1, 0:65] = xc[:, :, 1:130:2]
        in_maps.append({"xin": xpl, "cin": cst, "win": win, "gin": gin})
    return in_maps


def _gather(results):
    out = np.empty((N, C, HO, WO), np.float32)
    for n in range(N):
        d = np.asarray(results[n]["out"], np.float32)
        out[n, :, 0:HOC, :] = d[0:64]
        out[n, :, HOC:, :] = d[64:128]
    return out


def _enable_axon_trace():
    """Register the NTFF profile hook that this image's antenv lacks."""
    if _STATE.get("trace_hooked"):
        return
    import types
    import antenv
    from concourse import bass_utils
    mod = types.ModuleType("antenv.axon_hooks")
    mod._hook = None
    mod.set_axon_ntff_profile_hook = lambda h: setattr(mod, "_hook", h)
    mod.get_axon_ntff_profile_hook = lambda: mod._hook
    sys.modules["antenv.axon_hooks"] = mod
    antenv.axon_hooks = mod
    from trn_agent_boot.trn_boot import _ntff_profile_via_ctypes
    mod._hook = _ntff_profile_via_ctypes("/opt/axon/libaxon_pjrt.so")
    bass_utils.upload_artifacts = lambda tmpdir: tmpdir
    _STATE["trace_hooked"] = True


def run(x, conv_w, bn_gamma, bn_beta, bn_mean, bn_var, trace=False):
    from concourse.bass_utils import run_bass_kernel_spmd
    if trace:
        _enable_axon_trace()
    if "nc" not in _STATE:
        _STATE["nc"] = _build_bass()
    in_maps = _prep_inputs(x, conv_w, bn_gamma, bn_beta, bn_mean, bn_var)
    res = run_bass_kernel_spmd(_STATE["nc"], in_maps, list(range(N)), trace=trace)
    _STATE["last"] = res
    return _gather(res.results)


def kernel(x, conv_w, bn_gamma, bn_beta, bn_mean, bn_var):
    return run(x, conv_w, bn_gamma, bn_beta, bn_mean, bn_var,
               trace=bool(int(os.environ.get("KERNEL_TRACE", "0"))))
